# revision 1
# baseline (speedup 1.0000x reference)
"""A2N double-attention block (sparse_attention) on 8 TRN2 NeuronCores.

Reference computation (per full tensors):
    A  = w1 @ x + b1          [b, cm, hw]
    Bp = w2 @ x + b2          [b, cn, hw]
    V  = w3 @ x + b3          [b, cn, hw]
    att_maps = softmax(Bp, axis=0)   # over BATCH (torch implicit-dim rule)
    att_vecs = softmax(V,  axis=0)
    G  = einsum('bmp,bnp->bmn', A, att_maps)
    D  = einsum('bmn,bnp->bmp', G, att_vecs)
    y  = x + w4 @ D + b4

Sharding: spatial. Core k owns hw positions [k*512, (k+1)*512) for ALL 8
batches, so the batch-axis softmax is core-local (all 8 batch values of a
given (n, p) live on one core). The only cross-core term is G (gathered),
whose spatial contraction spans all cores -> one AllReduce of G^T
[b, n, m] partials, done in two fp16 chunks of 4 batches each so the
first chunk's AllReduce overlaps the second chunk's compute.

Layout notes (per core):
  - everything is kept "transposed" so every matmul contracts on the
    partition axis with zero on-chip transposes:
      A^T, Bp^T:   [p, m]/[p, n]  (p on partitions)
      V, att_vecs: [n, p]         (n on partitions)
      G^T:         [n, m]
  - b2/b3 cancel exactly in the batch softmax (same shift for every batch
    element) and are ignored. b4 is folded into x host-side. b1 is added
    on-device via a broadcast tile.
  - matmul inputs fp16 (fp32 matmul is 4 cycles/row on TRN2, fp16 is 1);
    all accumulation fp32 in PSUM; softmax denominators fp32.
"""

import sys

import numpy as np

if "/opt/trn_rl_repo" not in sys.path:
    sys.path.insert(0, "/opt/trn_rl_repo")

B, C, CM, CN = 8, 512, 512, 256
H = W = 64
HW = H * W
NCORES = 8
P = HW // NCORES  # spatial positions per core
HB = B // 2  # batches per AllReduce chunk

_cache = {}


def _build():
    import concourse.bacc as bacc
    import concourse.mybir as mybir
    import concourse.tile as tile

    dt = mybir.dt
    f16 = dt.float16
    f32 = dt.float32
    Exp = mybir.ActivationFunctionType.Exp
    add = mybir.AluOpType.add
    mult = mybir.AluOpType.mult

    CTn = C // 128  # contraction tiles over c
    PTn = P // 128  # tiles over local spatial p
    NTn = CN // 128  # tiles over n
    MTn = CM // 128  # tiles over m
    rg = [list(range(NCORES))]

    nc = bacc.Bacc("TRN2", target_bir_lowering=False, debug=False, num_devices=NCORES)

    xb_d = nc.dram_tensor("xb", [C, B, P], f16, kind="ExternalInput")
    w1t_d = nc.dram_tensor("w1t", [C, CM], f16, kind="ExternalInput")
    w2t_d = nc.dram_tensor("w2t", [C, CN], f16, kind="ExternalInput")
    w3t_d = nc.dram_tensor("w3t", [C, CN], f16, kind="ExternalInput")
    w4t_d = nc.dram_tensor("w4t", [CM, C], f16, kind="ExternalInput")
    b1bc_d = nc.dram_tensor("b1bc", [128, CM], f32, kind="ExternalInput")
    out_d = nc.dram_tensor("out", [C, B, P], f32, kind="ExternalOutput")

    with tile.TileContext(nc) as tc:
        with (
            tc.tile_pool(name="const", bufs=1) as cpool,
            tc.tile_pool(name="dram", bufs=1, space="DRAM") as dpool,
        ):
            xb = cpool.tile([128, CTn, B, P], f16)
            w1t = cpool.tile([128, CTn, CM], f16)
            w2t = cpool.tile([128, CTn, CN], f16)
            w3t = cpool.tile([128, CTn, CN], f16)
            w4t = cpool.tile([128, MTn, C], f16)
            b1bc = cpool.tile([128, CM], f32)
            E = cpool.tile([128, B, PTn, CN], f16)  # exp(Bp^T), then att_maps^T
            F = cpool.tile([128, B, NTn, P], f16)  # exp(V), then att_vecs
            denM = cpool.tile([128, PTn, CN], f32)
            denV = cpool.tile([128, NTn, P], f32)
            recM = cpool.tile([128, PTn, CN], f32)
            recV = cpool.tile([128, NTn, P], f32)
            Gar = cpool.tile([128, B, NTn, CM], f16)  # AllReduced G^T

            gin = [dpool.tile([HB, CN, CM], f16, name=f"gin{i}") for i in range(2)]
            gout = [
                dpool.tile([HB, CN, CM], f16, addr_space="Shared", name=f"gout{i}")
                for i in range(2)
            ]

            nc.sync.dma_start(w1t[:], w1t_d[:].rearrange("(t p) m -> p t m", p=128))
            nc.sync.dma_start(w2t[:], w2t_d[:].rearrange("(t p) m -> p t m", p=128))
            nc.sync.dma_start(w3t[:], w3t_d[:].rearrange("(t p) m -> p t m", p=128))
            nc.sync.dma_start(w4t[:], w4t_d[:].rearrange("(t p) m -> p t m", p=128))
            nc.sync.dma_start(b1bc[:], b1bc_d[:])
            xb_view = xb_d[:].rearrange("(t p) b q -> p t b q", p=128)
            for b in range(B):
                nc.sync.dma_start(xb[:, :, b, :], xb_view[:, :, b, :])

            # ---- Phase 1: Bp^T and V for every batch, softmax denominators.
            with (
                tc.tile_pool(name="ps_pb", bufs=2, space="PSUM") as pb_pool,
                tc.tile_pool(name="ps_v", bufs=2, space="PSUM") as v_pool,
            ):
                for b in range(B):
                    pb_ps = pb_pool.tile([128, PTn, CN], f32, tag="pb")
                    for pt in range(PTn):
                        for ct in range(CTn):
                            nc.tensor.matmul(
                                pb_ps[:, pt, :],
                                xb[:, ct, b, pt * 128 : (pt + 1) * 128],
                                w2t[:, ct, :],
                                start=(ct == 0),
                                stop=(ct == CTn - 1),
                            )
                    nc.scalar.activation(E[:, b, :, :], pb_ps[:], Exp)
                    if b == 0:
                        nc.vector.tensor_copy(denM[:], E[:, b, :, :])
                    else:
                        nc.vector.tensor_tensor(denM[:], denM[:], E[:, b, :, :], add)

                    v_ps = v_pool.tile([128, NTn, P], f32, tag="v")
                    for nt in range(NTn):
                        for ct in range(CTn):
                            nc.tensor.matmul(
                                v_ps[:, nt, :],
                                w3t[:, ct, nt * 128 : (nt + 1) * 128],
                                xb[:, ct, b, :],
                                start=(ct == 0),
                                stop=(ct == CTn - 1),
                            )
                    nc.scalar.activation(F[:, b, :, :], v_ps[:], Exp)
                    if b == 0:
                        nc.vector.tensor_copy(denV[:], F[:, b, :, :])
                    else:
                        nc.vector.tensor_tensor(denV[:], denV[:], F[:, b, :, :], add)

            # ---- Phase 2: reciprocals + att_maps normalization (in place).
            nc.vector.reciprocal(recM[:], denM[:])
            nc.vector.reciprocal(recV[:], denV[:])
            for b in range(B):
                nc.vector.tensor_tensor(E[:, b, :, :], E[:, b, :, :], recM[:], mult)

            # ---- Phase 3: A^T per batch, G^T partials, chunked AllReduce.
            with (
                tc.tile_pool(name="ps_at", bufs=1, space="PSUM") as at_pool,
                tc.tile_pool(name="ps_g", bufs=2, space="PSUM") as g_pool,
                tc.tile_pool(name="at_sb", bufs=2) as at_sb_pool,
                tc.tile_pool(name="gp_sb", bufs=4) as gp_pool,
            ):
                for b in range(B):
                    at_ps = at_pool.tile([128, PTn, CM], f32, tag="at")
                    for pt in range(PTn):
                        for ct in range(CTn):
                            nc.tensor.matmul(
                                at_ps[:, pt, :],
                                xb[:, ct, b, pt * 128 : (pt + 1) * 128],
                                w1t[:, ct, :],
                                start=(ct == 0),
                                stop=(ct == CTn - 1),
                            )
                    at_sb = at_sb_pool.tile([128, PTn, CM], f16, tag="at_sb")
                    for pt in range(PTn):
                        nc.vector.tensor_tensor(
                            at_sb[:, pt, :], at_ps[:, pt, :], b1bc[:], add
                        )
                    g_ps = g_pool.tile([128, NTn, CM], f32, tag="g")
                    for nt in range(NTn):
                        for pt in range(PTn):
                            nc.tensor.matmul(
                                g_ps[:, nt, :],
                                E[:, b, pt, nt * 128 : (nt + 1) * 128],
                                at_sb[:, pt, :],
                                start=(pt == 0),
                                stop=(pt == PTn - 1),
                            )
                    gp_sb = gp_pool.tile([128, NTn, CM], f16, tag="gp")
                    nc.scalar.copy(gp_sb[:], g_ps[:])
                    chunk, local = divmod(b, HB)
                    nc.sync.dma_start(
                        gin[chunk][local].rearrange("(t p) m -> p t m", p=128),
                        gp_sb[:],
                    )
                    if local == HB - 1:
                        nc.gpsimd.collective_compute(
                            "AllReduce",
                            add,
                            replica_groups=rg,
                            ins=[gin[chunk][:]],
                            outs=[gout[chunk][:]],
                        )
                        for lb in range(HB):
                            gb = chunk * HB + lb
                            nc.gpsimd.dma_start(
                                Gar[:, gb, :, :],
                                gout[chunk][lb].rearrange("(t p) m -> p t m", p=128),
                            )

            # ---- Phase 4: distributed, final conv, residual, store.
            with (
                tc.tile_pool(name="ps_d", bufs=1, space="PSUM") as d_pool,
                tc.tile_pool(name="ps_y", bufs=1, space="PSUM") as y_pool,
                tc.tile_pool(name="d_sb", bufs=2) as d_sb_pool,
                tc.tile_pool(name="y_sb", bufs=2) as y_sb_pool,
            ):
                out_view = out_d[:].rearrange("(t p) b q -> p t b q", p=128)
                # att_vecs normalization for batch 0 before its matmuls; the
                # rest are emitted one iteration ahead inside the loop so the
                # in-order DVE stream never stalls the PE on it.
                nc.vector.tensor_tensor(F[:, 0, :, :], F[:, 0, :, :], recV[:], mult)
                for b in range(B):
                    if b + 1 < B:
                        nc.vector.tensor_tensor(
                            F[:, b + 1, :, :], F[:, b + 1, :, :], recV[:], mult
                        )
                    d_ps = d_pool.tile([128, MTn, P], f32, tag="d")
                    for mt in range(MTn):
                        for nt in range(NTn):
                            nc.tensor.matmul(
                                d_ps[:, mt, :],
                                Gar[:, b, nt, mt * 128 : (mt + 1) * 128],
                                F[:, b, nt, :],
                                start=(nt == 0),
                                stop=(nt == NTn - 1),
                            )
                    d_sb = d_sb_pool.tile([128, MTn, P], f16, tag="d_sb")
                    nc.scalar.copy(d_sb[:], d_ps[:])
                    y_ps = y_pool.tile([128, CTn, P], f32, tag="y")
                    for ct in range(CTn):
                        for mt in range(MTn):
                            nc.tensor.matmul(
                                y_ps[:, ct, :],
                                w4t[:, mt, ct * 128 : (ct + 1) * 128],
                                d_sb[:, mt, :],
                                start=(mt == 0),
                                stop=(mt == MTn - 1),
                            )
                    y_sb = y_sb_pool.tile([128, CTn, P], f32, tag="y_sb")
                    nc.vector.tensor_tensor(y_sb[:], y_ps[:], xb[:, :, b, :], add)
                    nc.sync.dma_start(out_view[:, :, b, :], y_sb[:])

    nc.compile()
    return nc


def _get_nc():
    if "nc" not in _cache:
        _cache["nc"] = _build()
    return _cache["nc"]


def _prep_in_maps(x, w1, b1, w2, b2, w3, b3, w4, b4):
    x = np.asarray(x, dtype=np.float32).reshape(B, C, HW)
    b4 = np.asarray(b4, dtype=np.float32)
    # b4 folds into the residual input; b2/b3 cancel in the batch softmax.
    xt = (x + b4[None, :, None]).transpose(1, 0, 2).astype(np.float16)  # [C, B, HW]
    w1t = np.ascontiguousarray(np.asarray(w1, np.float32).T).astype(np.float16)
    w2t = np.ascontiguousarray(np.asarray(w2, np.float32).T).astype(np.float16)
    w3t = np.ascontiguousarray(np.asarray(w3, np.float32).T).astype(np.float16)
    w4t = np.ascontiguousarray(np.asarray(w4, np.float32).T).astype(np.float16)
    b1bc = np.ascontiguousarray(
        np.broadcast_to(np.asarray(b1, np.float32)[None, :], (128, CM))
    )
    in_maps = []
    for k in range(NCORES):
        in_maps.append(
            {
                "xb": np.ascontiguousarray(xt[:, :, k * P : (k + 1) * P]),
                "w1t": w1t,
                "w2t": w2t,
                "w3t": w3t,
                "w4t": w4t,
                "b1bc": b1bc,
            }
        )
    return in_maps


def _assemble(results):
    y = np.empty((B, C, HW), np.float32)
    for k in range(NCORES):
        y[:, :, k * P : (k + 1) * P] = results[k]["out"].transpose(1, 0, 2)
    return y.reshape(B, C, H, W)


def run(inputs, trace=False):
    """Run on hardware; returns (output, BassKernelResults)."""
    from concourse.bass_utils import run_bass_kernel_spmd

    nc = _get_nc()
    in_maps = _prep_in_maps(**inputs)
    res = run_bass_kernel_spmd(
        nc, in_maps, core_ids=list(range(NCORES)), trace=trace
    )
    return _assemble(res.results), res


def kernel(**inputs) -> np.ndarray:
    out, _ = run(inputs)
    return out


# revision 2
# speedup vs baseline: 1.2420x; 1.2420x over previous
"""A2N double-attention block (sparse_attention) on 8 TRN2 NeuronCores.

Reference computation (per full tensors):
    A  = w1 @ x + b1          [b, cm, hw]
    Bp = w2 @ x + b2          [b, cn, hw]
    V  = w3 @ x + b3          [b, cn, hw]
    att_maps = softmax(Bp, axis=0)   # over BATCH (torch implicit-dim rule)
    att_vecs = softmax(V,  axis=0)
    G  = einsum('bmp,bnp->bmn', A, att_maps)
    D  = einsum('bmn,bnp->bmp', G, att_vecs)
    y  = x + w4 @ D + b4

Sharding: spatial. Core k owns hw positions [k*512, (k+1)*512) for ALL 8
batches, so the batch-axis softmax is core-local. The only cross-core
term is G, whose spatial contraction spans all cores -> AllReduce of
G^T [b, n, m] fp16 partials in four chunks of 2 batches, pipelined
against the per-batch compute that produces/consumes them.

Layouts keep every matmul contraction on the partition axis with zero
on-chip transposes:
    A^T, Bp^T: [p, m]/[p, n];  V, att_vecs: [n, p];  G^T: [n, m]
b2/b3 cancel exactly in the batch softmax and are ignored. b4 is folded
into x host-side. b1 (zero in this problem's inputs) takes a DVE-add
fallback path when nonzero. Matmul operands fp16, accumulation fp32.
"""

import sys

import numpy as np

if "/opt/trn_rl_repo" not in sys.path:
    sys.path.insert(0, "/opt/trn_rl_repo")

B, C, CM, CN = 8, 512, 512, 256
H = W = 64
HW = H * W
NCORES = 8
P = HW // NCORES  # spatial positions per core
CHUNK = 2  # batches per AllReduce chunk
NCHUNKS = B // CHUNK
PF = 2  # A-block prefetch depth

_cache = {}


def _build(with_b1: bool):
    import concourse.bacc as bacc
    import concourse.mybir as mybir
    import concourse.tile as tile

    dt = mybir.dt
    f16 = dt.float16
    f32 = dt.float32
    Exp = mybir.ActivationFunctionType.Exp
    add = mybir.AluOpType.add
    mult = mybir.AluOpType.mult

    CTn = C // 128
    PTn = P // 128
    NTn = CN // 128
    MTn = CM // 128
    rg = [list(range(NCORES))]

    nc = bacc.Bacc("TRN2", target_bir_lowering=False, debug=False, num_devices=NCORES)

    xb_d = nc.dram_tensor("xb", [C, B, P], f16, kind="ExternalInput")
    w1t_d = nc.dram_tensor("w1t", [C, CM], f16, kind="ExternalInput")
    w2t_d = nc.dram_tensor("w2t", [C, CN], f16, kind="ExternalInput")
    w3t_d = nc.dram_tensor("w3t", [C, CN], f16, kind="ExternalInput")
    w4t_d = nc.dram_tensor("w4t", [CM, C], f16, kind="ExternalInput")
    b1bc_d = (
        nc.dram_tensor("b1bc", [128, CM], f32, kind="ExternalInput")
        if with_b1
        else None
    )
    out_d = nc.dram_tensor("out", [C, B, P], f32, kind="ExternalOutput")

    with tile.TileContext(nc) as tc:
        with (
            tc.tile_pool(name="const", bufs=1) as cpool,
            tc.tile_pool(name="dram", bufs=1, space="DRAM") as dpool,
        ):
            xb = cpool.tile([128, CTn, B, P], f16)
            w1t = cpool.tile([128, CTn, CM], f16)
            w2t = cpool.tile([128, CTn, CN], f16)
            w3t = cpool.tile([128, CTn, CN], f16)
            w4t = cpool.tile([128, MTn, C], f16)
            b1bc = cpool.tile([128, CM], f32) if with_b1 else None
            E = cpool.tile([128, B, PTn, CN], f16)  # exp(Bp^T) -> att_maps^T
            F = cpool.tile([128, B, NTn, P], f16)  # exp(V) -> att_vecs
            accM = cpool.tile([128, PTn, CN], f16)
            accV = cpool.tile([128, NTn, P], f16)
            denM = cpool.tile([128, PTn, CN], f32)
            denV = cpool.tile([128, NTn, P], f32)
            recM = cpool.tile([128, PTn, CN], f32)
            recV = cpool.tile([128, NTn, P], f32)
            Gar = cpool.tile([128, B, NTn, CM], f16)  # AllReduced G^T

            gin = [
                dpool.tile([CHUNK, CN, CM], f16, name=f"gin{i}")
                for i in range(NCHUNKS)
            ]
            gout = [
                dpool.tile([CHUNK, CN, CM], f16, addr_space="Shared", name=f"gout{i}")
                for i in range(NCHUNKS)
            ]

            # DMA order: what phase 1's first matmuls need comes first.
            xb_view = xb_d[:].rearrange("(t p) b q -> p t b q", p=128)
            nc.sync.dma_start(w2t[:], w2t_d[:].rearrange("(t p) m -> p t m", p=128))
            nc.sync.dma_start(w3t[:], w3t_d[:].rearrange("(t p) m -> p t m", p=128))
            for ct in range(CTn):
                nc.sync.dma_start(xb[:, ct, 0, :], xb_view[:, ct, 0, :])
            for b in range(1, B):
                nc.sync.dma_start(xb[:, :, b, :], xb_view[:, :, b, :])
            nc.sync.dma_start(w1t[:], w1t_d[:].rearrange("(t p) m -> p t m", p=128))
            nc.sync.dma_start(w4t[:], w4t_d[:].rearrange("(t p) m -> p t m", p=128))
            if with_b1:
                nc.sync.dma_start(b1bc[:], b1bc_d[:])

            # ---- Phase 1: Bp^T and V for every batch + softmax denominators.
            # Denominators accumulate in fp16 (fast DVE mode), final add fp32.
            with (
                tc.tile_pool(name="ps_pb", bufs=2, space="PSUM") as pb_pool,
                tc.tile_pool(name="ps_v", bufs=2, space="PSUM") as v_pool,
            ):
                for b in range(B):
                    pb_ps = pb_pool.tile([128, PTn, CN], f32, tag="pb")
                    for pt in range(PTn):
                        for ct in range(CTn):
                            nc.tensor.matmul(
                                pb_ps[:, pt, :],
                                xb[:, ct, b, pt * 128 : (pt + 1) * 128],
                                w2t[:, ct, :],
                                start=(ct == 0),
                                stop=(ct == CTn - 1),
                            )
                    nc.scalar.activation(E[:, b, :, :], pb_ps[:], Exp)
                    v_ps = v_pool.tile([128, NTn, P], f32, tag="v")
                    for nt in range(NTn):
                        for ct in range(CTn):
                            nc.tensor.matmul(
                                v_ps[:, nt, :],
                                w3t[:, ct, nt * 128 : (nt + 1) * 128],
                                xb[:, ct, b, :],
                                start=(ct == 0),
                                stop=(ct == CTn - 1),
                            )
                    nc.scalar.activation(F[:, b, :, :], v_ps[:], Exp)
                    if b == 1:
                        nc.vector.tensor_tensor(
                            accM[:], E[:, 0, :, :], E[:, 1, :, :], add
                        )
                        nc.vector.tensor_tensor(
                            accV[:], F[:, 0, :, :], F[:, 1, :, :], add
                        )
                    elif 1 < b < B - 1:
                        nc.vector.tensor_tensor(accM[:], accM[:], E[:, b, :, :], add)
                        nc.vector.tensor_tensor(accV[:], accV[:], F[:, b, :, :], add)
                    elif b == B - 1:
                        nc.vector.tensor_tensor(denM[:], accM[:], E[:, b, :, :], add)
                        nc.vector.tensor_tensor(denV[:], accV[:], F[:, b, :, :], add)

            # ---- Phase 2: reciprocals + normalizations, ordered by need.
            nc.vector.reciprocal_approx_fast(recM[:], denM[:])
            for b in range(2):
                nc.vector.tensor_tensor(E[:, b, :, :], E[:, b, :, :], recM[:], mult)
            nc.vector.reciprocal_approx_fast(recV[:], denV[:])
            for b in range(2):
                nc.vector.tensor_tensor(F[:, b, :, :], F[:, b, :, :], recV[:], mult)
            for b in range(2, B):
                nc.vector.tensor_tensor(E[:, b, :, :], E[:, b, :, :], recM[:], mult)
                nc.vector.tensor_tensor(F[:, b, :, :], F[:, b, :, :], recV[:], mult)

            # ---- Phase 3: A^T (prefetched), G^T partials, chunked AllReduce.
            with (
                tc.tile_pool(name="ps_at", bufs=2, space="PSUM") as at_pool,
                tc.tile_pool(name="ps_g", bufs=2, space="PSUM") as g_pool,
                tc.tile_pool(name="at_sb", bufs=PF + 1) as at_sb_pool,
                tc.tile_pool(name="gp_sb", bufs=2) as gp_pool,
            ):
                at_sbs = {}

                def emit_a(b):
                    at_sb = at_sb_pool.tile(
                        [128, PTn, CM], f16, tag="at_sb", name=f"at_sb{b}"
                    )
                    for h in range(2):
                        at_ps = at_pool.tile(
                            [128, 2, CM], f32, tag="at", name=f"at_ps{b}_{h}"
                        )
                        for pi in range(2):
                            pt = 2 * h + pi
                            for ct in range(CTn):
                                nc.tensor.matmul(
                                    at_ps[:, pi, :],
                                    xb[:, ct, b, pt * 128 : (pt + 1) * 128],
                                    w1t[:, ct, :],
                                    start=(ct == 0),
                                    stop=(ct == CTn - 1),
                                )
                        if with_b1:
                            for pi in range(2):
                                nc.vector.tensor_tensor(
                                    at_sb[:, 2 * h + pi, :],
                                    at_ps[:, pi, :],
                                    b1bc[:],
                                    add,
                                )
                        else:
                            nc.scalar.copy(at_sb[:, 2 * h : 2 * h + 2, :], at_ps[:])
                    at_sbs[b] = at_sb

                def emit_g(b):
                    g_ps = g_pool.tile([128, NTn, CM], f32, tag="g", name=f"g{b}")
                    for nt in range(NTn):
                        for pt in range(PTn):
                            nc.tensor.matmul(
                                g_ps[:, nt, :],
                                E[:, b, pt, nt * 128 : (nt + 1) * 128],
                                at_sbs[b][:, pt, :],
                                start=(pt == 0),
                                stop=(pt == PTn - 1),
                            )
                    gp_sb = gp_pool.tile([128, NTn, CM], f16, tag="gp", name=f"gp{b}")
                    nc.scalar.copy(gp_sb[:], g_ps[:])
                    chunk, local = divmod(b, CHUNK)
                    nc.sync.dma_start(
                        gin[chunk][local].rearrange("(t p) m -> p t m", p=128),
                        gp_sb[:],
                    )
                    if local == CHUNK - 1:
                        nc.gpsimd.collective_compute(
                            "AllReduce",
                            add,
                            replica_groups=rg,
                            ins=[gin[chunk][:]],
                            outs=[gout[chunk][:]],
                        )
                        for lb in range(CHUNK):
                            gb = chunk * CHUNK + lb
                            nc.gpsimd.dma_start(
                                Gar[:, gb, :, :],
                                gout[chunk][lb].rearrange("(t p) m -> p t m", p=128),
                            )

                for b in range(PF):
                    emit_a(b)
                for b in range(B):
                    emit_g(b)
                    if b + PF < B:
                        emit_a(b + PF)

            # ---- Phase 4: distributed, final conv (+residual), store.
            with (
                tc.tile_pool(name="ps_d", bufs=2, space="PSUM") as d_pool,
                tc.tile_pool(name="ps_y", bufs=2, space="PSUM") as y_pool,
                tc.tile_pool(name="d_sb", bufs=2) as d_sb_pool,
                tc.tile_pool(name="y_sb", bufs=2) as y_sb_pool,
            ):
                out_view = out_d[:].rearrange("(t p) b q -> p t b q", p=128)
                for b in range(B):
                    d_sb = d_sb_pool.tile([128, MTn, P], f16, tag="d_sb")
                    for h in range(2):
                        d_ps = d_pool.tile([128, 2, P], f32, tag="d", name=f"d{b}_{h}")
                        for mi in range(2):
                            mt = 2 * h + mi
                            for nt in range(NTn):
                                nc.tensor.matmul(
                                    d_ps[:, mi, :],
                                    Gar[:, b, nt, mt * 128 : (mt + 1) * 128],
                                    F[:, b, nt, :],
                                    start=(nt == 0),
                                    stop=(nt == NTn - 1),
                                )
                        nc.scalar.copy(d_sb[:, 2 * h : 2 * h + 2, :], d_ps[:])
                    y_pss = [
                        y_pool.tile([128, 2, P], f32, tag="y", name=f"y{b}_{h}")
                        for h in range(2)
                    ]
                    for mt in range(MTn):
                        for ct in range(CTn):
                            nc.tensor.matmul(
                                y_pss[ct // 2][:, ct % 2, :],
                                w4t[:, mt, ct * 128 : (ct + 1) * 128],
                                d_sb[:, mt, :],
                                start=(mt == 0),
                                stop=(mt == MTn - 1),
                            )
                    y_sb = y_sb_pool.tile([128, CTn, P], f32, tag="y_sb")
                    for h in range(2):
                        nc.vector.tensor_tensor(
                            y_sb[:, 2 * h : 2 * h + 2, :],
                            y_pss[h][:],
                            xb[:, 2 * h : 2 * h + 2, b, :],
                            add,
                        )
                    nc.sync.dma_start(out_view[:, :, b, :], y_sb[:])

    nc.compile()
    return nc


def _get_nc(with_b1: bool):
    key = ("nc", with_b1)
    if key not in _cache:
        _cache[key] = _build(with_b1)
    return _cache[key]


def _prep_in_maps(x, w1, b1, w2, b2, w3, b3, w4, b4, with_b1):
    x = np.asarray(x, dtype=np.float32).reshape(B, C, HW)
    b4 = np.asarray(b4, dtype=np.float32)
    # b4 folds into the residual input; b2/b3 cancel in the batch softmax.
    xt = (x + b4[None, :, None]).transpose(1, 0, 2).astype(np.float16)  # [C, B, HW]
    w1t = np.ascontiguousarray(np.asarray(w1, np.float32).T).astype(np.float16)
    w2t = np.ascontiguousarray(np.asarray(w2, np.float32).T).astype(np.float16)
    w3t = np.ascontiguousarray(np.asarray(w3, np.float32).T).astype(np.float16)
    w4t = np.ascontiguousarray(np.asarray(w4, np.float32).T).astype(np.float16)
    in_maps = []
    for k in range(NCORES):
        m = {
            "xb": np.ascontiguousarray(xt[:, :, k * P : (k + 1) * P]),
            "w1t": w1t,
            "w2t": w2t,
            "w3t": w3t,
            "w4t": w4t,
        }
        if with_b1:
            m["b1bc"] = np.ascontiguousarray(
                np.broadcast_to(np.asarray(b1, np.float32)[None, :], (128, CM))
            )
        in_maps.append(m)
    return in_maps


def _assemble(results):
    y = np.empty((B, C, HW), np.float32)
    for k in range(NCORES):
        y[:, :, k * P : (k + 1) * P] = results[k]["out"].transpose(1, 0, 2)
    return y.reshape(B, C, H, W)


def run(inputs, trace=False):
    """Run on hardware; returns (output, BassKernelResults)."""
    from concourse.bass_utils import run_bass_kernel_spmd

    with_b1 = bool(np.any(np.asarray(inputs["b1"]) != 0))
    nc = _get_nc(with_b1)
    in_maps = _prep_in_maps(**inputs, with_b1=with_b1)
    res = run_bass_kernel_spmd(nc, in_maps, core_ids=list(range(NCORES)), trace=trace)
    return _assemble(res.results), res


def kernel(**inputs) -> np.ndarray:
    out, _ = run(inputs)
    return out
